# revision 11
# baseline (speedup 1.0000x reference)
"""Distributed Trainium2 kernel for nn_AdjLayer (conv3x3 -> softmax -> outer(colsum)).

Strategy:
  - Host: im2col the tiny input (48400 x 28, incl. ones column for bias),
    shard 6050 pixels per core, pad to 6144, pack as [128, 1536] with 4
    pixel-groups at partition offsets 0/32/64/96 (28 im2col rows each).
  - Device (SPMD x8): per 128-pixel tile: matmul (K=28) -> PSUM [128,156],
    exp on ScalarE with accumulated row-sum, reciprocal + normalize on
    VectorE, running column-sum accumulator, stream S tiles to HBM.
  - Host: gather S shards, c = sum of per-core accumulators (minus the
    deterministic padding contribution), new_adj = outer(c, c).
"""

import sys

import numpy as np

for _p in ("/opt/trn_rl_repo",):
    if _p not in sys.path:
        sys.path.insert(0, _p)

N_F = 156          # filters
N_PIX = 48400      # 220*220 output pixels
N_CORES = 8
PER_CORE = N_PIX // N_CORES   # 6050
K = 28             # 27 conv taps + 1 ones-row (bias)
GROUPS = 3         # pixel groups at partition offsets 0/32/64
TILE = 128         # pixels per matmul tile
TILES_PER_G = 16
G_PIX = TILES_PER_G * TILE        # 1536 pixels per group
PIX_PAD = GROUPS * G_PIX          # 6144 padded pixels per core
N_PAD = PIX_PAD - PER_CORE        # 94 zero-im2col padding pixels per core

_GRAPH = None


def _build_graph():
    from concourse import bacc
    from concourse import mybir
    from concourse import tile

    f32 = mybir.dt.float32
    nc = bacc.Bacc(None)

    # single merged input: columns [0:G_PIX] = im2col^T, [G_PIX:] = weights
    xw_ext = nc.declare_dram_parameter(
        "xw", [128, G_PIX + N_F], f32, isOutput=False
    )
    # [48, 128, 156] is byte-identical to [6144, 156] (tile-major rows)
    n_tiles = GROUPS * TILES_PER_G
    s_ext = nc.declare_dram_parameter(
        "s", [n_tiles, TILE, N_F], f32, isOutput=True
    )
    acc_ext = nc.declare_dram_parameter("acc", [128, N_F], f32, isOutput=True)

    Exp = mybir.ActivationFunctionType.Exp

    with tile.TileContext(nc) as tc:
        with (
            tc.tile_pool(name="const", bufs=1) as cpool,
            tc.tile_pool(name="work", bufs=48) as wpool,
            tc.tile_pool(name="psum", bufs=8, space="PSUM") as ppool,
        ):
            xw_sb = cpool.tile([128, G_PIX + N_F], f32)
            acc_sb = cpool.tile([128, N_F], f32)
            sst = cpool.tile([128, n_tiles * N_F], f32)  # staged S, tile-major
            nc.sync.dma_start(out=xw_sb[:], in_=xw_ext[:])
            nc.vector.memset(acc_sb[:], 0.0)
            xT_sb = xw_sb[:, :G_PIX]
            w_sb = xw_sb[:, G_PIX:]

            CHUNK = 8  # tiles per output DMA (1+6+1 DMAs = 8 sem lanes)
            t = 0
            for g in range(GROUPS):
                p0 = 32 * g
                for j in range(TILES_PER_G):
                    psum_t = ppool.tile([TILE, N_F], f32, tag="psum")
                    nc.tensor.matmul(
                        psum_t[:],
                        lhsT=xT_sb[p0 : p0 + K, j * TILE : (j + 1) * TILE],
                        rhs=w_sb[p0 : p0 + K, :],
                        start=True,
                        stop=True,
                    )
                    exp_t = wpool.tile([TILE, N_F], f32, tag="exp")
                    sum_t = wpool.tile([TILE, 1], f32, tag="sum")
                    nc.scalar.activation(
                        exp_t[:], psum_t[:], Exp, accum_out=sum_t[:]
                    )
                    rec_t = wpool.tile([TILE, 1], f32, tag="rec")
                    nc.vector.reciprocal(rec_t[:], sum_t[:])
                    s_slice = sst[:, t * N_F : (t + 1) * N_F]
                    nc.vector.tensor_scalar_mul(s_slice, exp_t[:], rec_t[:])
                    nc.vector.tensor_add(acc_sb[:], acc_sb[:], s_slice)
                    if t % CHUNK == CHUNK - 1:
                        c0 = t - (CHUNK - 1)
                        dst = s_ext[c0 : c0 + CHUNK].rearrange(
                            "t p f -> p t f"
                        )
                        src = sst[
                            :, c0 * N_F : (c0 + CHUNK) * N_F
                        ].rearrange("p (t f) -> p t f", t=CHUNK)
                        nc.sync.dma_start(out=dst, in_=src)
                    t += 1
            nc.sync.dma_start(out=acc_ext[:], in_=acc_sb[:])

    nc.finalize()
    return nc


def _get_graph():
    global _GRAPH
    if _GRAPH is None:
        _GRAPH = _build_graph()
    return _GRAPH


def _prepare_inputs(inputs, W, b):
    """Host-side im2col + per-core packing."""
    from numpy.lib.stride_tricks import sliding_window_view

    x = np.ascontiguousarray(np.asarray(inputs, dtype=np.float32)[0])  # [222,222,3]
    W = np.asarray(W, dtype=np.float32)
    b = np.asarray(b, dtype=np.float32)

    # [220,220,3(c),3(dy),3(dx)] -> [y,x,dy,dx,c] -> [48400, 27]
    win = sliding_window_view(x, (3, 3), axis=(0, 1))
    cols = win.transpose(0, 1, 3, 4, 2).reshape(N_PIX, 27)
    cols = np.concatenate(
        [cols, np.ones((N_PIX, 1), dtype=np.float32)], axis=1
    )  # [48400, 28]

    w28 = np.concatenate([W.reshape(27, N_F), b[None, :]], axis=0)  # [28,156]

    in_maps = []
    for i in range(N_CORES):
        shard = cols[i * PER_CORE : (i + 1) * PER_CORE]
        pad = np.zeros((PIX_PAD, K), dtype=np.float32)
        pad[:PER_CORE] = shard
        xw = np.zeros((128, G_PIX + N_F), dtype=np.float32)
        for g in range(GROUPS):
            xw[32 * g : 32 * g + K, :G_PIX] = pad[g * G_PIX : (g + 1) * G_PIX].T
            xw[32 * g : 32 * g + K, G_PIX:] = w28
        in_maps.append({"xw": np.ascontiguousarray(xw)})
    return in_maps


def _run(inputs, W, b, trace=False):
    from concourse.bass_utils import run_bass_kernel_spmd

    in_maps = _prepare_inputs(inputs, W, b)
    nc = _get_graph()
    res = run_bass_kernel_spmd(
        nc, in_maps, core_ids=list(range(N_CORES)), trace=trace
    )

    S = np.empty((N_PIX, N_F), dtype=np.float32)
    c = np.zeros(N_F, dtype=np.float64)
    for i in range(N_CORES):
        S[i * PER_CORE : (i + 1) * PER_CORE] = res.results[i]["s"].reshape(
            PIX_PAD, N_F
        )[:PER_CORE]
        c += res.results[i]["acc"].sum(axis=0, dtype=np.float64)
    # each padding pixel contributed softmax(0-logits) = 1/156 to every column
    c -= N_CORES * N_PAD / N_F
    c = c.astype(np.float32)
    new_adj = np.outer(c, c).astype(np.float32)
    return (new_adj, S), res


def kernel(**inputs):
    (new_adj, S), _ = _run(inputs["inputs"], inputs["W"], inputs["b"])
    return (new_adj, S)


# revision 17
# speedup vs baseline: 1.5859x; 1.5859x over previous
"""Distributed Trainium2 kernel for nn_AdjLayer (conv3x3 -> softmax -> outer(colsum)).

Strategy:
  - Host: im2col the tiny input (48400 x 28, incl. ones column for bias) in
    bf16, shard 6050 pixels per core, pad to 6144, pack as [128, 2204] with
    3 pixel-groups at partition offsets 0/32/64 (28 im2col rows each) and
    the weights in the first 156 columns.
  - Device (SPMD x8), per 3-tile superblock (3 x 128 pixels):
      3 bf16 matmuls (K=28) -> one PSUM tile [128, 468]
      one wide Exp on ScalarE -> SBUF
      one 3D-AP row-sum reduce on VectorE -> per-pixel sumexp
      batched reciprocals (12 at a time) on VectorE
      one broadcast-multiply (stride-0 AP) on VectorE -> staged S
      one f32r ones-matmul on TensorE accumulating column sums into PSUM
    S streamed out in 6 chunked DMAs (8 tiles each).
  - Host: gather S shards, c from per-core accumulators (minus the
    deterministic padding contribution), new_adj = outer(c, c).
"""

import sys

import numpy as np

for _p in ("/opt/trn_rl_repo",):
    if _p not in sys.path:
        sys.path.insert(0, _p)

N_F = 156          # filters
N_PIX = 48400      # 220*220 output pixels
N_CORES = 8
PER_CORE = N_PIX // N_CORES   # 6050
K = 28             # 27 conv taps + 1 ones-row (bias)
GROUPS = 3         # pixel groups at partition offsets 0/32/64
TILE = 128         # pixels per matmul tile
TILES_PER_G = 16
G_PIX = TILES_PER_G * TILE        # 2048 pixels per group
PIX_PAD = GROUPS * G_PIX          # 6144 padded pixels per core
N_PAD = PIX_PAD - PER_CORE        # 94 zero-im2col padding pixels per core
N_TILES = GROUPS * TILES_PER_G    # 48
SB = 3                            # tiles per superblock
N_SB = N_TILES // SB              # 16
CHUNK = 8                         # tiles per output DMA (6 DMAs)
XW_COLS = N_F + G_PIX             # weights cols [0:156], im2col [156:2204]

_GRAPH = None


def _build_graph():
    from concourse import bacc
    from concourse import mybir
    from concourse import tile

    f32 = mybir.dt.float32
    bf16 = mybir.dt.bfloat16
    f32r = mybir.dt.float32r
    nc = bacc.Bacc(None)

    xw_ext = nc.declare_dram_parameter("xw", [128, XW_COLS], bf16, isOutput=False)
    # [48, 128, 156] is byte-identical to [6144, 156] (tile-major rows)
    s_ext = nc.declare_dram_parameter("s", [N_TILES, TILE, N_F], f32, isOutput=True)
    acc_ext = nc.declare_dram_parameter("acc", [1, N_F], f32, isOutput=True)

    Exp = mybir.ActivationFunctionType.Exp
    Copy = mybir.ActivationFunctionType.Copy
    X = mybir.AxisListType.X

    SPLIT = N_F + 8 * TILE  # input dma1 covers W + first 8 tiles of each group

    with tile.TileContext(nc) as tc:
        with (
            tc.tile_pool(name="const", bufs=1) as cpool,
            tc.tile_pool(name="work", bufs=8) as wpool,
            tc.tile_pool(name="psum", bufs=7, space="PSUM") as ppool,
            tc.tile_pool(name="psc", bufs=1, space="PSUM") as pcpool,
        ):
            xw_sb = cpool.tile([128, XW_COLS], bf16)
            sst = cpool.tile([128, N_TILES * N_F], f32)   # staged S
            sums_w = cpool.tile([128, N_TILES], f32)      # per-pixel sumexp
            recip_w = cpool.tile([128, N_TILES], f32)
            ones_sb = cpool.tile([128, 1], f32)
            csb = cpool.tile([1, SB * N_F], f32)
            acc_sb = cpool.tile([1, N_F], f32)

            nc.sync.dma_start(out=xw_sb[:, :SPLIT], in_=xw_ext[:, :SPLIT])
            nc.sync.dma_start(out=xw_sb[:, SPLIT:], in_=xw_ext[:, SPLIT:])
            nc.vector.memset(ones_sb[:], 1.0)

            psum_c = pcpool.tile([1, SB * N_F], f32)

            RB = 2  # superblocks per reciprocal batch (mul lags RB blocks)
            next_chunk = 0
            exp_tiles = {}
            for b in range(N_SB):
                psum_b = ppool.tile([TILE, SB * N_F], f32, tag="ps")
                for u in range(SB):
                    t = SB * b + u
                    g, j = divmod(t, TILES_PER_G)
                    p0 = 32 * g
                    nc.tensor.matmul(
                        psum_b[:, u * N_F : (u + 1) * N_F],
                        lhsT=xw_sb[p0 : p0 + K, N_F + j * TILE : N_F + (j + 1) * TILE],
                        rhs=xw_sb[p0 : p0 + K, :N_F],
                        start=True,
                        stop=True,
                    )
                exp_b = wpool.tile([TILE, SB * N_F], f32, tag="exp")
                exp_tiles[b] = exp_b
                nc.scalar.activation(exp_b[:], psum_b[:], Exp)
                nc.vector.reduce_sum(
                    out=sums_w[:, SB * b : SB * (b + 1)],
                    in_=exp_b.rearrange("p (t f) -> p t f", t=SB),
                    axis=X,
                )
                if b % RB != RB - 1:
                    continue
                q0 = SB * (b - RB + 1)
                nc.vector.reciprocal(
                    recip_w[:, q0 : q0 + RB * SB], sums_w[:, q0 : q0 + RB * SB]
                )
                for bb in range(b - RB + 1, b + 1):
                    s_slice = sst[:, SB * N_F * bb : SB * N_F * (bb + 1)]
                    rec_b = (
                        recip_w[:, SB * bb : SB * (bb + 1)]
                        .rearrange("p (t o) -> p t o", o=1)
                        .broadcast_to([128, SB, N_F])
                    )
                    nc.vector.tensor_mul(
                        s_slice.bitcast(f32r).rearrange("p (t f) -> p t f", t=SB),
                        exp_tiles.pop(bb).rearrange("p (t f) -> p t f", t=SB),
                        rec_b,
                    )
                    # accumulate column sums on the PE (f32r, one wide matmul)
                    nc.tensor.matmul(
                        psum_c[:],
                        lhsT=ones_sb[:].bitcast(f32r),
                        rhs=s_slice.bitcast(f32r),
                        start=(bb == 0),
                        stop=(bb == N_SB - 1),
                        skip_group_check=True,
                    )
                while (next_chunk + 1) * CHUNK <= SB * (b + 1):
                    t0 = next_chunk * CHUNK
                    dst = s_ext[t0 : t0 + CHUNK].rearrange("t p f -> p t f")
                    src = sst[
                        :, t0 * N_F : (t0 + CHUNK) * N_F
                    ].rearrange("p (t f) -> p t f", t=CHUNK)
                    nc.sync.dma_start(out=dst, in_=src)
                    next_chunk += 1

            nc.scalar.activation(csb[:], psum_c[:], Copy)
            nc.vector.tensor_add(
                acc_sb[:], csb[:, :N_F], csb[:, N_F : 2 * N_F]
            )
            nc.vector.tensor_add(
                acc_sb[:], acc_sb[:], csb[:, 2 * N_F : 3 * N_F]
            )
            nc.gpsimd.dma_start(out=acc_ext[:], in_=acc_sb[:])

    nc.finalize()
    return nc


def _get_graph():
    global _GRAPH
    if _GRAPH is None:
        _GRAPH = _build_graph()
    return _GRAPH


def _prepare_inputs(inputs, W, b):
    """Host-side im2col + per-core packing (bf16)."""
    import ml_dtypes
    from numpy.lib.stride_tricks import sliding_window_view

    x = np.ascontiguousarray(np.asarray(inputs, dtype=np.float32)[0])  # [222,222,3]
    W = np.asarray(W, dtype=np.float32)
    b = np.asarray(b, dtype=np.float32)

    # [220,220,3(c),3(dy),3(dx)] -> [y,x,dy,dx,c] -> [48400, 27]
    win = sliding_window_view(x, (3, 3), axis=(0, 1))
    cols = win.transpose(0, 1, 3, 4, 2).reshape(N_PIX, 27)
    cols = np.concatenate(
        [cols, np.ones((N_PIX, 1), dtype=np.float32)], axis=1
    )  # [48400, 28]

    w28 = np.concatenate([W.reshape(27, N_F), b[None, :]], axis=0)  # [28,156]

    in_maps = []
    for i in range(N_CORES):
        shard = cols[i * PER_CORE : (i + 1) * PER_CORE]
        pad = np.zeros((PIX_PAD, K), dtype=np.float32)
        pad[:PER_CORE] = shard
        xw = np.zeros((128, XW_COLS), dtype=np.float32)
        for g in range(GROUPS):
            xw[32 * g : 32 * g + K, :N_F] = w28
            xw[32 * g : 32 * g + K, N_F:] = pad[g * G_PIX : (g + 1) * G_PIX].T
        in_maps.append({"xw": xw.astype(ml_dtypes.bfloat16)})
    return in_maps


def _run(inputs, W, b, trace=False):
    from concourse.bass_utils import run_bass_kernel_spmd

    in_maps = _prepare_inputs(inputs, W, b)
    nc = _get_graph()
    res = run_bass_kernel_spmd(
        nc, in_maps, core_ids=list(range(N_CORES)), trace=trace
    )

    S = np.empty((N_PIX, N_F), dtype=np.float32)
    c = np.zeros(N_F, dtype=np.float64)
    for i in range(N_CORES):
        S[i * PER_CORE : (i + 1) * PER_CORE] = res.results[i]["s"].reshape(
            PIX_PAD, N_F
        )[:PER_CORE]
        c += res.results[i]["acc"][0].astype(np.float64)
    # each padding pixel contributed softmax(0-logits) = 1/156 to every column
    c -= N_CORES * N_PAD / N_F
    c = c.astype(np.float32)
    new_adj = np.outer(c, c).astype(np.float32)
    return (new_adj, S), res


def kernel(**inputs):
    (new_adj, S), _ = _run(inputs["inputs"], inputs["W"], inputs["b"])
    return (new_adj, S)
